# revision 21
# baseline (speedup 1.0000x reference)
# Trainium2 Bass kernel: nn_DecoderAttentionLayer (sliding-window decoder layer)
# Sequence-parallel over 8 NeuronCores: core = (n, quarter); each core processes
# 1024 tokens (+128-token halo for the previous key/value chunk).
#
# Software-pipelined over chunks: iteration c runs qkv(c) | o_proj(c-3) |
# transposes(c-1) | attention(c-2) so all engines stay fed.
#   - fp8 DoubleRow matmuls for qkv and o_proj (weights/x pre-scaled on host)
#   - all rsqrt via DVE Newton iteration (no ACT Sqrt -> no ACT table thrash)
#   - scores psum preloaded with -1e9 band mask; exp has accum_out = den
#   - y = o only (bf16); the x residual is added on the host
import sys
import numpy as np
import ml_dtypes

sys.path.insert(0, "/opt/trn_rl_repo")

import bass_rust
import concourse.bass as bass
import concourse.tile as tile
from concourse import mybir
from concourse.bass_utils import run_bass_kernel_spmd
from concourse.vector_clock import ScopedClock

F32 = mybir.dt.float32
BF16 = mybir.dt.bfloat16
FP8 = mybir.dt.float8e4
U32 = mybir.dt.uint32
AF = mybir.ActivationFunctionType
ALU = mybir.AluOpType
DR = mybir.MatmulPerfMode.DoubleRow
BF = ml_dtypes.bfloat16
F8 = ml_dtypes.float8_e4m3

N, T, D = 2, 4096, 1024
HD, NH, W = 64, 16, 128
EPS = 1.1920929e-07
TLOC = 1152          # 128 halo + 1024 own tokens
NCH = 9              # x chunks per core (chunk 0 = halo)
NPAIR = 8            # head pairs

# fp8 scale folding: x fed as 16x, w as 64w -> psum = 1024 * true
XS = 16.0
WS = 64.0
MMS = XS * WS
VS = 8.0             # v stored as 8 * v_true
OWS = 4096.0         # o_proj weight scale
MB = -1.0e9          # additive mask bias


def _split_excess_waits(nc):
    cnt = 0
    for f in nc.m.functions:
        for b in f.blocks:
            changed = False
            new_insts = []
            for inst in b.instructions:
                si = inst.sync_info
                waits = list(si.on_wait) if (si is not None and si.on_wait) else []
                if len(waits) > 1:
                    si.on_wait = waits[:1]
                    for w in waits[1:]:
                        cnt += 1
                        nop = bass_rust.InstNoOp(
                            name=f"I-waitfix-{cnt}", engine=inst.engine)
                        nop.sync_info = mybir.SyncInfo(on_wait=[w], on_update=[])
                        new_insts.append(nop)
                    changed = True
                new_insts.append(inst)
            if changed:
                b.instructions = new_insts
    return cnt


def _patched_drain_and_barrier(self, tick_clock, wait_clock):
    drain_inst = self.nc.sync.drain()
    wait_clock.add_sem_waits(
        drain_inst.ins, ScopedClock({None: tick_clock.global_clock}))
    si = drain_inst.ins.sync_info
    if si is not None and si.on_wait and len(si.on_wait) > 1:
        waits = list(si.on_wait)
        si.on_wait = waits[:1]
        for w in waits[1:]:
            extra = self.nc.sync.drain()
            esi = extra.ins.sync_info
            if esi is None:
                extra.ins.sync_info = mybir.SyncInfo(on_wait=[w], on_update=[])
            else:
                esi.on_wait = [w]
    self.nc.all_engine_barrier()
    assert self.sems is not None
    popped = self.nc._tile_sem_poison_stack.pop()
    assert popped is self._sem_poison
    self.nc.clear_and_free_semaphores(list(self.sems.allocated().values()))
    self.nc.all_engine_barrier()


tile.TileContext._drain_and_barrier = _patched_drain_and_barrier


def _ap(t, offset, dims):
    return bass.AP(tensor=t.tensor, offset=t.offset + offset, ap=[t.ap[0]] + dims)


def build_program(waitfix=True):
    nc = bass.Bass()

    x_nat = nc.dram_tensor("x_nat", [TLOC, D], BF16, kind="ExternalInput")
    xT2 = nc.dram_tensor("xT2", [128, 4, 2, TLOC], FP8, kind="ExternalInput")
    wT2 = nc.dram_tensor("wT2", [128, 4, 2, 3 * D], FP8, kind="ExternalInput")
    ow2 = nc.dram_tensor("ow2", [128, 4, 2, D], FP8, kind="ExternalInput")
    rot = nc.dram_tensor("rot", [TLOC, 64], BF16, kind="ExternalInput")
    maskF = nc.dram_tensor("maskF", [W, 4 * W], BF16, kind="ExternalInput")
    maskR = nc.dram_tensor("maskR", [W, 4 * W], BF16, kind="ExternalInput")
    eye = nc.dram_tensor("eye", [128, 128], BF16, kind="ExternalInput")
    y = nc.dram_tensor("y", [1024, D], BF16, kind="ExternalOutput")

    with tile.TileContext(nc) as tc:
        with tc.tile_pool(name="persist", bufs=1) as P, \
             tc.tile_pool(name="xpool", bufs=2) as XP, \
             tc.tile_pool(name="cspool", bufs=2) as CS, \
             tc.tile_pool(name="qkpool", bufs=3) as QK, \
             tc.tile_pool(name="small", bufs=4) as SM, \
             tc.tile_pool(name="probs", bufs=5) as PR, \
             tc.tile_pool(name="ypool", bufs=2) as YP, \
             tc.tile_pool(name="ps_uni", bufs=4, space="PSUM") as UNI, \
             tc.tile_pool(name="ps_tr", bufs=2, space="PSUM") as PST, \
             tc.tile_pool(name="ps_u", bufs=2, space="PSUM") as PSU:

            # ---------------- persistent loads ----------------
            xT_k = []
            for k2 in range(4):
                t = P.tile([128, 2, TLOC], FP8, tag=f"xT{k2}")
                nc.sync.dma_start(out=t, in_=xT2[:, k2, :, :])
                xT_k.append(t)
            wT_k = []
            for k2 in range(4):
                t = P.tile([128, 2, 3 * D], FP8, tag=f"wT{k2}")
                nc.sync.dma_start(out=t, in_=wT2[:, k2, :, :])
                wT_k.append(t)
            eye_t = P.tile([128, 128], BF16, tag="eye")
            nc.sync.dma_start(out=eye_t, in_=eye[:, :])
            mF = P.tile([W, 4 * W], BF16, tag="mF")
            nc.sync.dma_start(out=mF, in_=maskF[:, :])
            mR = P.tile([W, 4 * W], BF16, tag="mR")
            nc.sync.dma_start(out=mR, in_=maskR[:, :])
            ow_k = []
            for k2 in range(4):
                t = P.tile([128, 2, D], FP8, tag=f"ow{k2}")
                nc.sync.dma_start(out=t, in_=ow2[:, k2, :, :])
                ow_k.append(t)
            magic = P.tile([128, 1], U32, tag="magic")
            nc.vector.memset(magic, 0x5F3759DF)

            # persistent big activation stores
            qT_all = P.tile([128, NPAIR * TLOC], BF16, tag="qT_all")
            kT_all = P.tile([128, NPAIR * TLOC], BF16, tag="kT_all")
            attn_T = P.tile([128, NPAIR * 1024], FP8, tag="attn_T")
            inv_all = P.tile([128, NCH], F32, tag="inv_all")
            sq_all = P.tile([128, NCH * NH], F32, tag="sq_all")
            v_all = P.tile([128, NCH * 1024], BF16, tag="v_all")

            raws = {}

            def rsqrt_newton(y, m, ncols):
                """y = m ** -0.5 elementwise via quake seed + 2 Newton steps."""
                t_full = SM.tile([128, 33], F32, tag="nt_t")
                t = t_full[:, 0:ncols]
                nc.vector.tensor_scalar(
                    out=y.bitcast(U32), in0=m.bitcast(U32), scalar1=1,
                    scalar2=None, op0=ALU.logical_shift_right)
                nc.vector.tensor_tensor(
                    out=y.bitcast(U32), in0=_ap(magic, 0, [[0, ncols]]),
                    in1=y.bitcast(U32), op=ALU.subtract)
                for _ in range(2):
                    nc.vector.tensor_mul(t, y, y)
                    nc.vector.tensor_mul(t, m, t)
                    nc.vector.tensor_scalar(
                        out=t, in0=t, scalar1=-0.5, scalar2=1.5,
                        op0=ALU.mult, op1=ALU.add)
                    nc.vector.tensor_mul(y, y, t)

            def x_load(c):
                xt = XP.tile([128, D], BF16, tag="x")
                nc.sync.dma_start(out=xt, in_=x_nat[c * 128:(c + 1) * 128, :])
                return xt

            def x_stats(xt):
                """bn-stats of an x chunk -> msq tile [128,1] (mean(x^2)+eps)."""
                bstats = SM.tile([128, 2, 6], F32, tag="bstats")
                for g in range(2):
                    nc.vector.bn_stats(out=bstats[:, g, :],
                                       in_=xt[:, g * 512:(g + 1) * 512])
                mv = SM.tile([128, 2], F32, tag="mv")
                nc.vector.bn_aggr(out=mv, in_=bstats)
                msq = SM.tile([128, 1], F32, tag="msq")
                nc.vector.tensor_mul(msq, mv[:, 0:1], mv[:, 0:1])
                nc.vector.tensor_add(msq, msq, mv[:, 1:2])
                nc.vector.tensor_scalar_add(msq, msq, EPS)
                return msq

            # prologue: inv for chunk 0
            msq0 = x_stats(x_load(0))
            rsqrt_newton(inv_all[:, 0:1], msq0, 1)

            def a_mm(c):
                """qkv matmuls + v evac + q/k stats/scale/rotary + next x-rms."""
                xt_next = x_load(c + 1) if c + 1 < NCH else None
                cs = CS.tile([128, 64], BF16, tag="cs")
                nc.sync.dma_start(out=cs, in_=rot[c * 128:(c + 1) * 128, :])
                inv = inv_all[:, c:c + 1]
                inv2 = SM.tile([128, 1], F32, tag="inv2")
                nc.vector.tensor_mul(inv2, inv, inv)
                inv_v = SM.tile([128, 1], F32, tag="inv_v")
                nc.vector.tensor_scalar_mul(inv_v, inv, VS / MMS)

                def qkv_mm(jlo):
                    pss = []
                    for half in range(2):
                        ps = UNI.tile([128, 512], F32, tag="ps")
                        for k2 in range(4):
                            nc.tensor.matmul(
                                ps,
                                xT_k[k2][:, :, c * 128:(c + 1) * 128],
                                wT_k[k2][:, :,
                                         jlo + half * 512: jlo + (half + 1) * 512],
                                start=(k2 == 0), stop=(k2 == 3),
                                perf_mode=DR)
                        pss.append(ps)
                    return pss

                v_ps = qkv_mm(2048)
                for half in range(2):
                    nc.scalar.activation(
                        out=v_all[:, c * 1024 + half * 512:
                                  c * 1024 + (half + 1) * 512],
                        in_=v_ps[half], func=AF.Copy, scale=inv_v)

                which_list = (("k", 1024),) if c == 0 else (("q", 0), ("k", 1024))
                nt = SM.tile([128, 33], F32, tag="nt")
                rr = SM.tile([128, 33], F32, tag="rr")
                raw_wh = {}
                for which, jlo in which_list:
                    ps = qkv_mm(jlo)
                    raw = QK.tile([128, 1024], BF16, tag=f"{which}raw")
                    for half in range(2):
                        nc.scalar.copy(raw[:, half * 512:(half + 1) * 512],
                                       ps[half])
                    raws[(which, c)] = raw
                    raw_wh[which] = raw
                    sq = QK.tile([128, 1024], BF16, tag="sq")
                    nc.gpsimd.tensor_mul(sq, raw, raw)
                    ssq = SM.tile([128, NH], F32, tag="ssq")
                    nc.vector.tensor_reduce(
                        out=ssq, in_=sq.rearrange("p (h d) -> p h d", h=NH),
                        axis=mybir.AxisListType.X, op=ALU.add)
                    lo = 1 if which == "q" else 17
                    nc.vector.tensor_scalar(
                        out=nt[:, lo:lo + NH], in0=ssq, scalar1=inv2,
                        scalar2=1.0 / 64.0 / (MMS * MMS),
                        op0=ALU.mult, op1=ALU.mult)
                    nc.vector.tensor_scalar_add(
                        nt[:, lo:lo + NH], nt[:, lo:lo + NH], EPS)

                # x-rms for the next chunk rides along in column 0
                if c + 1 < NCH:
                    msq_n = x_stats(xt_next)
                    nc.vector.tensor_copy(nt[:, 0:1], msq_n)
                else:
                    nc.vector.memset(nt[:, 0:1], 1.0)
                if c == 0:
                    nc.vector.memset(nt[:, 1:17], 1.0)
                rsqrt_newton(rr, nt, 33)
                if c + 1 < NCH:
                    nc.vector.tensor_copy(inv_all[:, c + 1:c + 2], rr[:, 0:1])

                for which, jlo in which_list:
                    raw = raw_wh[which]
                    lo = 1 if which == "q" else 17
                    if which == "q":
                        scl = sq_all[:, c * NH:(c + 1) * NH]
                        nc.vector.tensor_scalar(
                            out=scl, in0=rr[:, lo:lo + NH], scalar1=inv,
                            scalar2=0.125 / MMS, op0=ALU.mult, op1=ALU.mult)
                    else:
                        scl = SM.tile([128, NH], F32, tag="scl")
                        nc.vector.tensor_scalar(
                            out=scl, in0=rr[:, lo:lo + NH], scalar1=inv,
                            scalar2=1.0 / MMS, op0=ALU.mult, op1=ALU.mult)
                        for h in range(NH):
                            nc.vector.tensor_scalar_mul(
                                out=raw[:, h * HD:(h + 1) * HD],
                                in0=raw[:, h * HD:(h + 1) * HD],
                                scalar1=scl[:, h:h + 1])
                    # rotary: t1 = swapped-half * (+/-sin); raw = raw*cos + t1
                    qs = QK.tile([128, NH, 2, 16], BF16, tag="qs")
                    nc.vector.tensor_copy(
                        qs, _ap(raw, 32, [[64, NH], [-32, 2], [1, 16]]))
                    t1 = QK.tile([128, NH, 2, 16], BF16, tag="t1")
                    nc.vector.tensor_mul(
                        t1, qs, _ap(cs, 32, [[0, NH], [16, 2], [1, 16]]))
                    act = _ap(raw, 0, [[64, NH], [32, 2], [1, 16]])
                    nc.gpsimd.tensor_mul(
                        act, act, _ap(cs, 0, [[0, NH], [16, 2], [1, 16]]))
                    nc.gpsimd.tensor_add(act, act, t1)

            def a_tr(c):
                whiches = ("k",) if c == 0 else ("q", "k")
                for which in whiches:
                    raw = raws.pop((which, c))
                    dst = qT_all if which == "q" else kT_all
                    for grp in range(2):
                        tp = PST.tile([128, 512], BF16, tag="tp")
                        for i in range(4):
                            p = grp * 4 + i
                            nc.tensor.transpose(
                                tp[:, i * 128:(i + 1) * 128],
                                raw[:, p * 128:(p + 1) * 128], eye_t)
                        nc.any.tensor_copy(
                            _ap(dst, (grp * 4) * TLOC + c * 128,
                                [[TLOC, 4], [1, 128]]), tp)

            def b_attn(c):
                mask = mF if c == 1 else mR
                for grp in range(2):
                    u_ps = PSU.tile([128, 512], F32, tag="u_ps")
                    for i in range(4):
                        p = grp * 4 + i
                        s_ps = []
                        for hh in range(2):
                            sp = UNI.tile([128, 512], F32, tag="ps")
                            nc.vector.tensor_copy(
                                sp[:, 0:256], mask[:, hh * 256:(hh + 1) * 256])
                            off = p * TLOC
                            nc.tensor.matmul(
                                sp[:, 0:256],
                                qT_all[hh * 64:(hh + 1) * 64,
                                       off + c * 128: off + (c + 1) * 128],
                                kT_all[hh * 64:(hh + 1) * 64,
                                       off + (c - 1) * 128: off + (c + 1) * 128],
                                start=False, stop=True)
                            s_ps.append(sp)
                        e_sb = PR.tile([128, 512], BF16, tag="e_sb")
                        den = SM.tile([128, 2], F32, tag="den")
                        for hh in range(2):
                            h = 2 * p + hh
                            nc.scalar.activation(
                                out=e_sb[:, hh * 256:(hh + 1) * 256],
                                in_=s_ps[hh][:, 0:256],
                                func=AF.Exp,
                                scale=sq_all[:, c * NH + h: c * NH + h + 1],
                                accum_out=den[:, hh:hh + 1])
                        invd = SM.tile([128, 2], F32, tag="invd")
                        nc.vector.reciprocal(out=invd, in_=den)
                        for hh in range(2):
                            nc.vector.tensor_scalar_mul(
                                out=e_sb[:, hh * 256:(hh + 1) * 256],
                                in0=e_sb[:, hh * 256:(hh + 1) * 256],
                                scalar1=invd[:, hh:hh + 1])
                        ptp = PST.tile([128, 512], BF16, tag="tp")
                        for i4 in range(4):
                            nc.tensor.transpose(
                                ptp[:, i4 * 128:(i4 + 1) * 128],
                                e_sb[:, i4 * 128:(i4 + 1) * 128], eye_t)
                        pT = PR.tile([128, 512], BF16, tag="pT")
                        nc.vector.tensor_copy(pT, ptp)
                        for hh in range(2):
                            h = 2 * p + hh
                            for kc in range(2):
                                nc.tensor.matmul(
                                    u_ps[hh * 64:(hh + 1) * 64,
                                         i * 128:(i + 1) * 128],
                                    v_all[:, (c - 1 + kc) * 1024 + h * 64:
                                          (c - 1 + kc) * 1024 + (h + 1) * 64],
                                    pT[:, (2 * hh + kc) * 128:
                                          (2 * hh + kc + 1) * 128],
                                    start=(kc == 0), stop=(kc == 1),
                                    tile_position=(0, hh * 64))
                    nc.scalar.activation(
                        out=_ap(attn_T, (grp * 4) * 1024 + (c - 1) * 128,
                                [[1024, 4], [1, 128]]),
                        in_=u_ps, func=AF.Copy, scale=1.0 / VS)

            def c_oproj(c):
                for half in range(2):
                    o_ps = UNI.tile([128, 512], F32, tag="ps")
                    for k2 in range(4):
                        nc.tensor.matmul(
                            o_ps,
                            _ap(attn_T, k2 * 2048 + (c - 1) * 128,
                                [[1024, 2], [1, 128]]),
                            ow_k[k2][:, :, half * 512:(half + 1) * 512],
                            start=(k2 == 0), stop=(k2 == 3),
                            perf_mode=DR)
                    yt = YP.tile([128, 512], BF16, tag="y")
                    nc.scalar.activation(
                        out=yt, in_=o_ps, func=AF.Copy, scale=1.0 / OWS)
                    nc.sync.dma_start(
                        out=y[(c - 1) * 128:c * 128,
                              half * 512:(half + 1) * 512], in_=yt)

            # ---------------- software pipeline ----------------
            for c in range(NCH + 3):
                if c <= 8:
                    a_mm(c)
                if 1 <= c - 3 <= 8:
                    c_oproj(c - 3)
                if 0 <= c - 1 <= 8:
                    a_tr(c - 1)
                if 1 <= c - 2 <= 8:
                    b_attn(c - 2)

    if waitfix:
        _split_excess_waits(nc)
    return nc


_PROGRAM = None


def _get_program():
    global _PROGRAM
    if _PROGRAM is None:
        _PROGRAM = build_program()
    return _PROGRAM


def _f8(a):
    return np.clip(a, -240.0, 240.0).astype(F8)


def _host_inputs(input_NTD, qkv_weight, o_weight, o_scale):
    x = np.asarray(input_NTD, dtype=np.float32)
    wq = np.asarray(qkv_weight, dtype=np.float32).reshape(3 * D, D)
    wT2 = _f8((wq.T * WS).reshape(4, 2, 128, 3 * D).transpose(2, 0, 1, 3))
    ows = np.asarray(o_weight, dtype=np.float32) * \
        np.asarray(o_scale, dtype=np.float32)[:, None]
    ow2 = _f8((ows.T * OWS).reshape(4, 2, 128, D).transpose(2, 0, 1, 3))
    eye = np.eye(128, dtype=np.float32).astype(BF)

    j = np.arange(W)[:, None]
    m = np.arange(2 * W)[None, :]
    base = (m > j) & (m <= W + j)
    maskR = np.where(base, 0.0, MB).astype(np.float32)
    maskF0 = np.where(base & (m >= W), 0.0, MB).astype(np.float32)
    maskR = np.concatenate([maskR, maskR], axis=1).astype(BF)
    maskF0 = np.concatenate([maskF0, maskF0], axis=1).astype(BF)

    freqs = (1.0 / 10000.0) ** np.linspace(0.0, 1.0, 16).astype(np.float32)

    in_maps = []
    for core in range(8):
        n, qq = divmod(core, 4)
        lo = qq * 1024 - 128
        if qq == 0:
            xs = np.concatenate(
                [np.zeros((128, D), np.float32), x[n, 0:1024]], axis=0)
        else:
            xs = x[n, lo:lo + 1024 + 128]
        xs = np.ascontiguousarray(xs)
        xT2 = _f8((xs.T * XS).reshape(4, 2, 128, TLOC).transpose(2, 0, 1, 3))
        pos = np.maximum(np.arange(lo, lo + TLOC), 0).astype(np.float32)
        theta = pos[:, None] * freqs[None, :]
        cos16, sin16 = np.cos(theta), np.sin(theta)
        rotc = np.concatenate(
            [cos16, cos16, sin16, -sin16], axis=1).astype(BF)
        in_maps.append(dict(
            x_nat=xs.astype(BF),
            xT2=xT2, wT2=wT2, ow2=ow2, rot=np.ascontiguousarray(rotc),
            maskF=(maskF0 if qq == 0 else maskR), maskR=maskR,
            eye=eye))
    return in_maps


def kernel(input_NTD, qkv_weight, o_weight, o_scale, _trace=False):
    nc = _get_program()
    in_maps = _host_inputs(input_NTD, qkv_weight, o_weight, o_scale)
    res = run_bass_kernel_spmd(nc, in_maps, core_ids=list(range(8)),
                               trace=_trace)
    kernel.last_results = res
    x = np.asarray(input_NTD, dtype=np.float32)
    out = np.empty((N, T, D), dtype=np.float32)
    for core in range(8):
        n, qq = divmod(core, 4)
        sl = slice(qq * 1024, (qq + 1) * 1024)
        out[n, sl] = x[n, sl] + res.results[core]["y"].astype(np.float32)
    return out


# revision 27
# speedup vs baseline: 1.2297x; 1.2297x over previous
# Trainium2 Bass kernel: nn_DecoderAttentionLayer (sliding-window decoder layer)
# Sequence-parallel over 8 NeuronCores: core = (n, quarter); each core processes
# 1024 tokens (+128-token halo for the previous key/value chunk).
#
# Software-pipelined over chunks: iteration c runs qkv(c) | o_proj(c-3) |
# transposes(c-1) | attention(c-2) so all engines stay fed.
#   - fp8 DoubleRow matmuls for qkv and o_proj (weights/x pre-scaled on host)
#   - all rsqrt via DVE Newton iteration (no ACT Sqrt -> no ACT table thrash)
#   - scores psum preloaded with -1e9 band mask; exp has accum_out = den
#   - y = o only (bf16); the x residual is added on the host
import sys
import numpy as np
import ml_dtypes

sys.path.insert(0, "/opt/trn_rl_repo")

import bass_rust
import concourse.bass as bass
import concourse.tile as tile
from concourse import mybir
from concourse.bass_utils import run_bass_kernel_spmd
from concourse.vector_clock import ScopedClock

F32 = mybir.dt.float32
BF16 = mybir.dt.bfloat16
FP8 = mybir.dt.float8e4
U32 = mybir.dt.uint32
AF = mybir.ActivationFunctionType
ALU = mybir.AluOpType
DR = mybir.MatmulPerfMode.DoubleRow
BF = ml_dtypes.bfloat16
F8 = ml_dtypes.float8_e4m3

N, T, D = 2, 4096, 1024
HD, NH, W = 64, 16, 128
EPS = 1.1920929e-07
TLOC = 1152          # 128 halo + 1024 own tokens
NCH = 9              # x chunks per core (chunk 0 = halo)
NPAIR = 8            # head pairs

# fp8 scale folding: x fed as 16x, w as 64w -> psum = 1024 * true
XS = 16.0
WS = 64.0
MMS = XS * WS
VS = 8.0             # v stored as 8 * v_true
OWS = 4096.0         # o_proj weight scale
MB = -1.0e9          # additive mask bias


def _split_excess_waits(nc):
    cnt = 0
    for f in nc.m.functions:
        for b in f.blocks:
            changed = False
            new_insts = []
            for inst in b.instructions:
                si = inst.sync_info
                waits = list(si.on_wait) if (si is not None and si.on_wait) else []
                if len(waits) > 1:
                    si.on_wait = waits[:1]
                    for w in waits[1:]:
                        cnt += 1
                        nop = bass_rust.InstNoOp(
                            name=f"I-waitfix-{cnt}", engine=inst.engine)
                        nop.sync_info = mybir.SyncInfo(on_wait=[w], on_update=[])
                        new_insts.append(nop)
                    changed = True
                new_insts.append(inst)
            if changed:
                b.instructions = new_insts
    return cnt


def _patched_drain_and_barrier(self, tick_clock, wait_clock):
    drain_inst = self.nc.sync.drain()
    wait_clock.add_sem_waits(
        drain_inst.ins, ScopedClock({None: tick_clock.global_clock}))
    si = drain_inst.ins.sync_info
    if si is not None and si.on_wait and len(si.on_wait) > 1:
        waits = list(si.on_wait)
        si.on_wait = waits[:1]
        for w in waits[1:]:
            extra = self.nc.sync.drain()
            esi = extra.ins.sync_info
            if esi is None:
                extra.ins.sync_info = mybir.SyncInfo(on_wait=[w], on_update=[])
            else:
                esi.on_wait = [w]
    self.nc.all_engine_barrier()
    assert self.sems is not None
    popped = self.nc._tile_sem_poison_stack.pop()
    assert popped is self._sem_poison
    self.nc.clear_and_free_semaphores(list(self.sems.allocated().values()))
    self.nc.all_engine_barrier()


tile.TileContext._drain_and_barrier = _patched_drain_and_barrier


def _ap(t, offset, dims):
    return bass.AP(tensor=t.tensor, offset=t.offset + offset, ap=[t.ap[0]] + dims)


def build_program(waitfix=True):
    nc = bass.Bass()

    x_nat = nc.dram_tensor("x_nat", [TLOC, D], BF16, kind="ExternalInput")
    xT2 = nc.dram_tensor("xT2", [128, 4, 2, TLOC], FP8, kind="ExternalInput")
    wT2 = nc.dram_tensor("wT2", [128, 4, 2, 3 * D], FP8, kind="ExternalInput")
    ow2 = nc.dram_tensor("ow2", [128, 4, 2, D], FP8, kind="ExternalInput")
    rot = nc.dram_tensor("rot", [TLOC, 64], BF16, kind="ExternalInput")
    maskF = nc.dram_tensor("maskF", [W, 4 * W], BF16, kind="ExternalInput")
    maskR = nc.dram_tensor("maskR", [W, 4 * W], BF16, kind="ExternalInput")
    eye = nc.dram_tensor("eye", [128, 128], BF16, kind="ExternalInput")
    y = nc.dram_tensor("y", [1024, D], BF16, kind="ExternalOutput")

    with tile.TileContext(nc) as tc:
        with tc.tile_pool(name="persist", bufs=1) as P, \
             tc.tile_pool(name="xpool", bufs=2) as XP, \
             tc.tile_pool(name="cspool", bufs=2) as CS, \
             tc.tile_pool(name="qkpool", bufs=3) as QK, \
             tc.tile_pool(name="small", bufs=4) as SM, \
             tc.tile_pool(name="probs", bufs=5) as PR, \
             tc.tile_pool(name="ypool", bufs=2) as YP, \
             tc.tile_pool(name="ps_uni", bufs=4, space="PSUM") as UNI, \
             tc.tile_pool(name="ps_tr", bufs=2, space="PSUM") as PST, \
             tc.tile_pool(name="ps_u", bufs=2, space="PSUM") as PSU:

            # ---------------- persistent loads ----------------
            xT_k = []
            for k2 in range(4):
                t = P.tile([128, 2, TLOC], FP8, tag=f"xT{k2}")
                nc.sync.dma_start(out=t, in_=xT2[:, k2, :, :])
                xT_k.append(t)
            wT_k = []
            for k2 in range(4):
                t = P.tile([128, 2, 3 * D], FP8, tag=f"wT{k2}")
                nc.sync.dma_start(out=t, in_=wT2[:, k2, :, :])
                wT_k.append(t)
            eye_t = P.tile([128, 128], BF16, tag="eye")
            nc.sync.dma_start(out=eye_t, in_=eye[:, :])
            mF = P.tile([W, 4 * W], BF16, tag="mF")
            nc.sync.dma_start(out=mF, in_=maskF[:, :])
            mR = P.tile([W, 4 * W], BF16, tag="mR")
            nc.sync.dma_start(out=mR, in_=maskR[:, :])
            ow_k = []
            for k2 in range(4):
                t = P.tile([128, 2, D], FP8, tag=f"ow{k2}")
                nc.sync.dma_start(out=t, in_=ow2[:, k2, :, :])
                ow_k.append(t)
            magic = P.tile([128, 1], U32, tag="magic")
            nc.vector.memset(magic, 0x5F3759DF)

            # persistent big activation stores
            qT_all = P.tile([128, NPAIR * TLOC], BF16, tag="qT_all")
            kT_all = P.tile([128, NPAIR * TLOC], BF16, tag="kT_all")
            attn_T = P.tile([128, NPAIR * 1024], FP8, tag="attn_T")
            inv_all = P.tile([128, NCH], F32, tag="inv_all")
            sq_all = P.tile([128, NCH * NH], F32, tag="sq_all")
            v_all = P.tile([128, NCH * 1024], BF16, tag="v_all")

            raws = {}

            def rsqrt_newton(y, m, ncols):
                """y = m ** -0.5 elementwise via quake seed + 1 Newton step."""
                t_full = SM.tile([128, 33], F32, tag="nt_t")
                t = t_full[:, 0:ncols]
                nc.vector.tensor_scalar(
                    out=y.bitcast(U32), in0=m.bitcast(U32), scalar1=1,
                    scalar2=None, op0=ALU.logical_shift_right)
                nc.vector.tensor_tensor(
                    out=y.bitcast(U32), in0=_ap(magic, 0, [[0, ncols]]),
                    in1=y.bitcast(U32), op=ALU.subtract)
                for _ in range(1):
                    nc.vector.tensor_mul(t, y, y)
                    nc.vector.tensor_mul(t, m, t)
                    nc.vector.tensor_scalar(
                        out=t, in0=t, scalar1=-0.5, scalar2=1.5,
                        op0=ALU.mult, op1=ALU.add)
                    nc.vector.tensor_mul(y, y, t)

            def x_load(c):
                xt = XP.tile([128, D], BF16, tag="x")
                nc.sync.dma_start(out=xt, in_=x_nat[c * 128:(c + 1) * 128, :])
                return xt

            def x_stats(xt, msq):
                """bn-stats of an x chunk -> msq [128,1] = mean(x^2)."""
                bstats = SM.tile([128, 2, 6], F32, tag="bstats")
                for g in range(2):
                    nc.vector.bn_stats(out=bstats[:, g, :],
                                       in_=xt[:, g * 512:(g + 1) * 512])
                mv = SM.tile([128, 2], F32, tag="mv")
                nc.vector.bn_aggr(out=mv, in_=bstats)
                nc.vector.tensor_mul(msq, mv[:, 0:1], mv[:, 0:1])
                nc.vector.tensor_add(msq, msq, mv[:, 1:2])
                nc.vector.tensor_scalar_add(msq, msq, EPS)

            # prologue: inv for chunk 0
            msq0 = SM.tile([128, 1], F32, tag="msq")
            x_stats(x_load(0), msq0)
            rsqrt_newton(inv_all[:, 0:1], msq0, 1)

            def a_mm(c):
                """qkv matmuls + v evac + q/k stats/scale/rotary + next x-rms."""
                xt_next = x_load(c + 1) if c + 1 < NCH else None
                cs = CS.tile([128, 64], BF16, tag="cs")
                nc.sync.dma_start(out=cs, in_=rot[c * 128:(c + 1) * 128, :])
                inv = inv_all[:, c:c + 1]
                inv2 = SM.tile([128, 1], F32, tag="inv2")
                nc.vector.tensor_mul(inv2, inv, inv)
                inv_v = SM.tile([128, 1], F32, tag="inv_v")
                nc.vector.tensor_scalar_mul(inv_v, inv, VS / MMS)

                def qkv_mm(jlo):
                    pss = []
                    for half in range(2):
                        ps = UNI.tile([128, 512], F32, tag="ps")
                        for k2 in range(4):
                            nc.tensor.matmul(
                                ps,
                                xT_k[k2][:, :, c * 128:(c + 1) * 128],
                                wT_k[k2][:, :,
                                         jlo + half * 512: jlo + (half + 1) * 512],
                                start=(k2 == 0), stop=(k2 == 3),
                                perf_mode=DR)
                        pss.append(ps)
                    return pss

                v_ps = qkv_mm(2048)
                for half in range(2):
                    nc.scalar.activation(
                        out=v_all[:, c * 1024 + half * 512:
                                  c * 1024 + (half + 1) * 512],
                        in_=v_ps[half], func=AF.Copy, scale=inv_v)

                which_list = (("k", 1024),) if c == 0 else (("q", 0), ("k", 1024))
                nt = SM.tile([128, 33], F32, tag="nt")
                rr = SM.tile([128, 33], F32, tag="rr")
                raw_wh = {}
                for which, jlo in which_list:
                    ps = qkv_mm(jlo)
                    raw = QK.tile([128, 1024], BF16, tag=f"{which}raw")
                    for half in range(2):
                        nc.scalar.copy(raw[:, half * 512:(half + 1) * 512],
                                       ps[half])
                    raws[(which, c)] = raw
                    raw_wh[which] = raw
                    sq = QK.tile([128, 1024], BF16, tag="sq")
                    nc.gpsimd.tensor_mul(sq, raw, raw)
                    ssq = SM.tile([128, NH], F32, tag="ssq")
                    nc.vector.tensor_reduce(
                        out=ssq, in_=sq.rearrange("p (h d) -> p h d", h=NH),
                        axis=mybir.AxisListType.X, op=ALU.add)
                    lo = 1 if which == "q" else 17
                    nc.vector.tensor_scalar(
                        out=nt[:, lo:lo + NH], in0=ssq, scalar1=inv2,
                        scalar2=1.0 / 64.0 / (MMS * MMS),
                        op0=ALU.mult, op1=ALU.mult)

                # x-rms for the next chunk rides along in column 0
                if c + 1 < NCH:
                    x_stats(xt_next, nt[:, 0:1])
                else:
                    nc.vector.memset(nt[:, 0:1], 1.0)
                if c == 0:
                    nc.vector.memset(nt[:, 1:17], 1.0)
                rsqrt_newton(rr, nt, 33)
                if c + 1 < NCH:
                    nc.vector.tensor_copy(inv_all[:, c + 1:c + 2], rr[:, 0:1])

                for which, jlo in which_list:
                    raw = raw_wh[which]
                    lo = 1 if which == "q" else 17
                    if which == "q":
                        scl = sq_all[:, c * NH:(c + 1) * NH]
                        nc.vector.tensor_scalar(
                            out=scl, in0=rr[:, lo:lo + NH], scalar1=inv,
                            scalar2=0.125 / MMS, op0=ALU.mult, op1=ALU.mult)
                    else:
                        scl = SM.tile([128, NH], F32, tag="scl")
                        nc.vector.tensor_scalar(
                            out=scl, in0=rr[:, lo:lo + NH], scalar1=inv,
                            scalar2=1.0 / MMS, op0=ALU.mult, op1=ALU.mult)
                        nc.vector.tensor_mul(
                            raw.rearrange("p (h d) -> p h d", h=NH),
                            raw.rearrange("p (h d) -> p h d", h=NH),
                            _ap(scl, 0, [[1, NH], [0, HD]]))
                    # rotary: t1 = swapped-half * (+/-sin); raw = raw*cos + t1
                    qs = QK.tile([128, NH, 2, 16], BF16, tag="qs")
                    nc.vector.tensor_copy(
                        qs, _ap(raw, 32, [[64, NH], [-32, 2], [1, 16]]))
                    t1 = QK.tile([128, NH, 2, 16], BF16, tag="t1")
                    nc.vector.tensor_mul(
                        t1, qs, _ap(cs, 32, [[0, NH], [16, 2], [1, 16]]))
                    act = _ap(raw, 0, [[64, NH], [32, 2], [1, 16]])
                    nc.gpsimd.tensor_mul(
                        act, act, _ap(cs, 0, [[0, NH], [16, 2], [1, 16]]))
                    nc.gpsimd.tensor_add(act, act, t1)

            def a_tr(c):
                whiches = ("k",) if c == 0 else ("q", "k")
                for which in whiches:
                    raw = raws.pop((which, c))
                    dst = qT_all if which == "q" else kT_all
                    for grp in range(2):
                        tp = PST.tile([128, 512], BF16, tag="tp")
                        for i in range(4):
                            p = grp * 4 + i
                            nc.tensor.transpose(
                                tp[:, i * 128:(i + 1) * 128],
                                raw[:, p * 128:(p + 1) * 128], eye_t)
                        nc.any.tensor_copy(
                            _ap(dst, (grp * 4) * TLOC + c * 128,
                                [[TLOC, 4], [1, 128]]), tp)

            def b_attn(c):
                mask = mF if c == 1 else mR
                for grp in range(2):
                    u_ps = PSU.tile([128, 512], F32, tag="u_ps")
                    for i in range(4):
                        p = grp * 4 + i
                        s_ps = []
                        for hh in range(2):
                            sp = UNI.tile([128, 512], F32, tag="ps")
                            nc.vector.tensor_copy(
                                sp[:, 0:256], mask[:, hh * 256:(hh + 1) * 256])
                            off = p * TLOC
                            nc.tensor.matmul(
                                sp[:, 0:256],
                                qT_all[hh * 64:(hh + 1) * 64,
                                       off + c * 128: off + (c + 1) * 128],
                                kT_all[hh * 64:(hh + 1) * 64,
                                       off + (c - 1) * 128: off + (c + 1) * 128],
                                start=False, stop=True)
                            s_ps.append(sp)
                        e_sb = PR.tile([128, 512], BF16, tag="e_sb")
                        den = SM.tile([128, 2], F32, tag="den")
                        for hh in range(2):
                            h = 2 * p + hh
                            nc.scalar.activation(
                                out=e_sb[:, hh * 256:(hh + 1) * 256],
                                in_=s_ps[hh][:, 0:256],
                                func=AF.Exp,
                                scale=sq_all[:, c * NH + h: c * NH + h + 1],
                                accum_out=den[:, hh:hh + 1])
                        invd = SM.tile([128, 2], F32, tag="invd")
                        nc.vector.reciprocal(out=invd, in_=den)
                        for hh in range(2):
                            nc.vector.tensor_scalar_mul(
                                out=e_sb[:, hh * 256:(hh + 1) * 256],
                                in0=e_sb[:, hh * 256:(hh + 1) * 256],
                                scalar1=invd[:, hh:hh + 1])
                        ptp = PST.tile([128, 512], BF16, tag="tp")
                        for i4 in range(4):
                            nc.tensor.transpose(
                                ptp[:, i4 * 128:(i4 + 1) * 128],
                                e_sb[:, i4 * 128:(i4 + 1) * 128], eye_t)
                        pT = PR.tile([128, 512], BF16, tag="pT")
                        nc.vector.tensor_copy(pT, ptp)
                        for hh in range(2):
                            h = 2 * p + hh
                            for kc in range(2):
                                nc.tensor.matmul(
                                    u_ps[hh * 64:(hh + 1) * 64,
                                         i * 128:(i + 1) * 128],
                                    v_all[:, (c - 1 + kc) * 1024 + h * 64:
                                          (c - 1 + kc) * 1024 + (h + 1) * 64],
                                    pT[:, (2 * hh + kc) * 128:
                                          (2 * hh + kc + 1) * 128],
                                    start=(kc == 0), stop=(kc == 1),
                                    tile_position=(0, hh * 64))
                    nc.scalar.activation(
                        out=_ap(attn_T, (grp * 4) * 1024 + (c - 1) * 128,
                                [[1024, 4], [1, 128]]),
                        in_=u_ps, func=AF.Copy, scale=1.0 / VS)

            def c_oproj(c):
                for half in range(2):
                    o_ps = UNI.tile([128, 512], F32, tag="ps")
                    for k2 in range(4):
                        nc.tensor.matmul(
                            o_ps,
                            _ap(attn_T, k2 * 2048 + (c - 1) * 128,
                                [[1024, 2], [1, 128]]),
                            ow_k[k2][:, :, half * 512:(half + 1) * 512],
                            start=(k2 == 0), stop=(k2 == 3),
                            perf_mode=DR)
                    yt = YP.tile([128, 512], BF16, tag="y")
                    nc.scalar.activation(
                        out=yt, in_=o_ps, func=AF.Copy, scale=1.0 / OWS)
                    nc.sync.dma_start(
                        out=y[(c - 1) * 128:c * 128,
                              half * 512:(half + 1) * 512], in_=yt)

            # ---------------- software pipeline ----------------
            for c in range(NCH + 3):
                if c <= 8:
                    a_mm(c)
                if 1 <= c - 3 <= 8:
                    c_oproj(c - 3)
                if 0 <= c - 1 <= 8:
                    a_tr(c - 1)
                if 1 <= c - 2 <= 8:
                    b_attn(c - 2)

    if waitfix:
        _split_excess_waits(nc)
    return nc


_PROGRAM = None


def _get_program():
    global _PROGRAM
    if _PROGRAM is None:
        _PROGRAM = build_program()
    return _PROGRAM


def _f8(a):
    return np.clip(a, -240.0, 240.0).astype(F8)


def _host_inputs(input_NTD, qkv_weight, o_weight, o_scale):
    x = np.asarray(input_NTD, dtype=np.float32)
    wq = np.asarray(qkv_weight, dtype=np.float32).reshape(3 * D, D)
    wT2 = _f8((wq.T * WS).reshape(4, 2, 128, 3 * D).transpose(2, 0, 1, 3))
    ows = np.asarray(o_weight, dtype=np.float32) * \
        np.asarray(o_scale, dtype=np.float32)[:, None]
    ow2 = _f8((ows.T * OWS).reshape(4, 2, 128, D).transpose(2, 0, 1, 3))
    eye = np.eye(128, dtype=np.float32).astype(BF)

    j = np.arange(W)[:, None]
    m = np.arange(2 * W)[None, :]
    base = (m > j) & (m <= W + j)
    maskR = np.where(base, 0.0, MB).astype(np.float32)
    maskF0 = np.where(base & (m >= W), 0.0, MB).astype(np.float32)
    maskR = np.concatenate([maskR, maskR], axis=1).astype(BF)
    maskF0 = np.concatenate([maskF0, maskF0], axis=1).astype(BF)

    freqs = (1.0 / 10000.0) ** np.linspace(0.0, 1.0, 16).astype(np.float32)

    in_maps = []
    for core in range(8):
        n, qq = divmod(core, 4)
        lo = qq * 1024 - 128
        if qq == 0:
            xs = np.concatenate(
                [np.zeros((128, D), np.float32), x[n, 0:1024]], axis=0)
        else:
            xs = x[n, lo:lo + 1024 + 128]
        xs = np.ascontiguousarray(xs)
        xT2 = _f8((xs.T * XS).reshape(4, 2, 128, TLOC).transpose(2, 0, 1, 3))
        pos = np.maximum(np.arange(lo, lo + TLOC), 0).astype(np.float32)
        theta = pos[:, None] * freqs[None, :]
        cos16, sin16 = np.cos(theta), np.sin(theta)
        rotc = np.concatenate(
            [cos16, cos16, sin16, -sin16], axis=1).astype(BF)
        in_maps.append(dict(
            x_nat=xs.astype(BF),
            xT2=xT2, wT2=wT2, ow2=ow2, rot=np.ascontiguousarray(rotc),
            maskF=(maskF0 if qq == 0 else maskR), maskR=maskR,
            eye=eye))
    return in_maps


def kernel(input_NTD, qkv_weight, o_weight, o_scale, _trace=False):
    nc = _get_program()
    in_maps = _host_inputs(input_NTD, qkv_weight, o_weight, o_scale)
    res = run_bass_kernel_spmd(nc, in_maps, core_ids=list(range(8)),
                               trace=_trace)
    kernel.last_results = res
    x = np.asarray(input_NTD, dtype=np.float32)
    out = np.empty((N, T, D), dtype=np.float32)
    for core in range(8):
        n, qq = divmod(core, 4)
        sl = slice(qq * 1024, (qq + 1) * 1024)
        out[n, sl] = x[n, sl] + res.results[core]["y"].astype(np.float32)
    return out
